# revision 3
# baseline (speedup 1.0000x reference)
"""OHEM cross-entropy loss kernel for Trainium2 (8 NeuronCores, Bass/Tile).

Math (matches reference.py):
    logp   = log_softmax(seg_logit, axis=1)          # [B,C,H,W], C=19
    x_l    = logp at label (ignore 255 -> class 0)
    prob   = exp(x_l)
    thr    = max(sort(prob.flatten())[MIN_KEPT*B], 0.7)
    loss   = mean(-x_l * (prob < thr))

Device strategy (data-parallel over B across 8 cores, one image per core).
The loss is a global mean over pixels (pixel order irrelevant) and the 2e-2
harness tolerance admits aggressive per-term approximation as long as the
per-pixel errors are mean-zero (4.2M pixels average them away; validated
~2e-4 end to end on the target distribution). Per pixel the device needs
x_l and lse = ln(sum_c exp(x_c)). Five engines split the work:

  - Host applies a PER-PIXEL class transposition (slot0 <-> label), so the
    label logit always sits at class-slot 0: the 19-way gather disappears
    entirely (sumexp is permutation-invariant).
  - DMA: 15 class slots staged fp8-e3m4 (1B), 4 slots f16 (slot0 = x_l
    needs the precision; the other 3 feed the DVE fast path directly)
    -> 23 B/pixel instead of 38.
  - ACT (scalar): native exp on the 9 fp8 "a"-slots (1 elem/cycle
    @1.2GHz, dtype-independent); its only other job. One table load.
  - GPSIMD: upconverts the 6 fp8 "c"-slots to f16 (1-input copy is near
    line rate) so the DVE can consume them in its 4x mode.
  - DVE (vector): Schraudolph bit-trick exp for the 10 b+c slots --
    ONE tensor_scalar per group: bits16(e^x) ~= round(x*1477.32 + B0),
    written as int16 and bitcast f16 (tensor_scalar/ tensor_copy are the
    only 4x-mode ops: 0.26 ns/elem f16; tensor_tensor is 2x, everything
    else 1x). Mean bit-error zeroed via B0 -= 58.7 (chord-vs-curve of
    2^f). Also computes the whole reduction tail (below).
  - PE (tensor): the 19->1 class sum runs as 19 accumulating identity
    matmuls into one PSUM bank ([128,F] f32 += eb[:,k,:]): PSUM is the
    128-lane accumulator, 1 column/cycle @2.4GHz, zero DVE cost.
  - lse via a second Schraudolph on the PSUM bits (DVE scalar_tensor_
    tensor): w = bits32(sumexp)*ln2/2^23 - (x_l + 127*ln2) = (lse - x_l)
    - CLN + sawtooth, CLN=0.0397 zeroing the mantissa-chord mean. Then
    min(v,C0)/count(v<C0) fold into max(w,M0)/count(w>M0) and two 4x
    tensor_scalar accumulations finish the chunk.

    Per-chunk tails are issued TWO CHUNKS LATE so the in-order DVE queue
    never waits on the PE accumulation of the current chunk.

    Host combines partials, falling back to an exact host path if
    count <= MIN_KEPT*B (never for the target distribution).
"""

import numpy as np
import ml_dtypes

B = 8
C = 19
H, W = 512, 1024
HW = H * W            # 524288 pixels per image/core
P = 128               # SBUF partitions
FREE = HW // P        # 4096 pixels per partition
# head/tail chunks smaller: cuts pipeline-fill and drain latency.
# F <= 512 (one PSUM bank / max moving free dim per matmul).
CHUNKS = [256, 512, 512, 512, 512, 512, 512, 512, 256]
assert sum(CHUNKS) == FREE
FMAX = max(CHUNKS)
NCHUNK = len(CHUNKS)

NB = 4                # f16 slots (slot 0 = label logit), DVE Schraudolph
NC_ = 6               # fp8 slots upconverted on GPSIMD, DVE Schraudolph
NA = C - NB - NC_     # 9 fp8 slots, native exp on ACT
# eb slot layout: [b 0:NB | c NB:NB+NC_ | a NB+NC_:C]
# lb8 row layout: [a 0:NA | c NA:NA+NC_]

C0 = float(np.log(np.float32(0.7)))
AS = 1477.3196        # 1024*log2(e)
B0 = 15360.0 - 58.7   # f16 exponent bias 15<<10, minus mean chord error
K1 = float(np.log(2.0) / 2**23)
SH = float(127 * np.log(2.0))   # 88.0296919: folds the f32 exponent bias
CLN = 0.0397          # mean of ln(1+f) - f*ln2 over the mantissa chord
M0 = float(-C0 - CLN)
MIN_KEPT = 100000
IGNORE_INDEX = 255
N_TOTAL = B * HW
LAG = 2               # chunks between PSUM production and its DVE tail

_CACHE = {}


def _build_nc():
    import concourse.bacc as bacc
    import concourse.mybir as mybir
    import concourse.tile as tile

    fp16 = mybir.dt.float16
    fp32 = mybir.dt.float32
    fp8 = mybir.dt.float8e3
    i16 = mybir.dt.int16
    i32 = mybir.dt.int32
    Alu = mybir.AluOpType

    nc = bacc.Bacc()
    # chunk-blocked layouts: per partition, chunk j's [rows, F_j] block is
    # contiguous, so each chunk DMA is 128 large contiguous dram reads
    logit8 = nc.dram_tensor("logit8", [P, (NA + NC_) * FREE], fp8,
                            kind="ExternalInput")
    logitb = nc.dram_tensor("logitb", [P, NB * FREE], fp16,
                            kind="ExternalInput")
    ident = nc.dram_tensor("ident", [P, P], fp16, kind="ExternalInput")
    acc = nc.dram_tensor("acc", [P, 2 * NCHUNK], fp32, kind="ExternalOutput")

    N8 = NA + NC_

    with tile.TileContext(nc) as tc:
        with (
            tc.tile_pool(name="lb8", bufs=3) as lb8_pool,
            tc.tile_pool(name="lbb", bufs=4) as lbb_pool,
            tc.tile_pool(name="conv", bufs=2) as conv_pool,
            tc.tile_pool(name="eb", bufs=3) as eb_pool,
            tc.tile_pool(name="ps", bufs=3, space="PSUM") as ps_pool,
            tc.tile_pool(name="sm", bufs=2) as sm_pool,
            tc.tile_pool(name="one", bufs=1) as one_pool,
        ):
            acc_t = one_pool.tile([P, 2 * NCHUNK], fp32)
            id_t = one_pool.tile([P, P], fp16)

            def emit_tail(j, f, lbb, ps):
                # xls = x_l + 127*ln2 (4x); w = bits32(sumexp)*K1 - xls (1x);
                # then the two 4x accumulations. min(v,C0) = -(max(w,M0)+CLN),
                # [v<C0] = [w>M0]; host undoes the constants.
                xls = sm_pool.tile([P, FMAX], fp16, tag="xls")
                nc.vector.tensor_scalar_add(
                    out=xls[:, 0:f], in0=lbb[:, 0, 0:f], scalar1=SH,
                )
                w = sm_pool.tile([P, FMAX], fp16, tag="w")
                nc.vector.scalar_tensor_tensor(
                    out=w[:, 0:f], in0=ps[:, 0:f].bitcast(i32), scalar=K1,
                    in1=xls[:, 0:f], op0=Alu.mult, op1=Alu.subtract,
                )
                scr = sm_pool.tile([P, FMAX], fp16, tag="scr")
                nc.vector.tensor_scalar(
                    out=scr[:, 0:f], in0=w[:, 0:f], scalar1=M0, scalar2=None,
                    op0=Alu.max, op1=Alu.add,
                    accum_out=acc_t[:, j : j + 1],
                )
                scr2 = sm_pool.tile([P, FMAX], fp16, tag="scr2")
                nc.vector.tensor_scalar(
                    out=scr2[:, 0:f], in0=w[:, 0:f], scalar1=M0, scalar2=None,
                    op0=Alu.is_gt, op1=Alu.add,
                    accum_out=acc_t[:, NCHUNK + j : NCHUNK + j + 1],
                )

            pending = []  # (j, F, lbb, ps) awaiting their reduction tail
            off = 0
            for j, F in enumerate(CHUNKS):
                lb8 = lb8_pool.tile([P, N8, FMAX], fp8, tag="lb8")
                nc.sync.dma_start(
                    out=lb8[:, :, 0:F],
                    in_=logit8[:, N8 * off : N8 * (off + F)].rearrange(
                        "p (c f) -> p c f", c=N8
                    ),
                )
                lbb = lbb_pool.tile([P, NB, FMAX], fp16, tag="lbb")
                nc.sync.dma_start(
                    out=lbb[:, :, 0:F],
                    in_=logitb[:, NB * off : NB * (off + F)].rearrange(
                        "p (c f) -> p c f", c=NB
                    ),
                )
                if j == 0:
                    nc.sync.dma_start(out=id_t[:], in_=ident[:, :])

                eb = eb_pool.tile([P, C, FMAX], fp16, tag="eb")
                # GPSIMD: upconvert the c-slots for the DVE 4x path
                conv = conv_pool.tile([P, NC_, FMAX], fp16, tag="conv")
                nc.gpsimd.tensor_copy(
                    out=conv[:, :, 0:F], in_=lb8[:, NA : NA + NC_, 0:F]
                )
                # DVE: Schraudolph exp, one 4x op per group
                nc.vector.tensor_scalar(
                    out=eb[:, 0:NB, 0:F].bitcast(i16), in0=lbb[:, :, 0:F],
                    scalar1=AS, scalar2=B0, op0=Alu.mult, op1=Alu.add,
                )
                nc.vector.tensor_scalar(
                    out=eb[:, NB : NB + NC_, 0:F].bitcast(i16),
                    in0=conv[:, :, 0:F],
                    scalar1=AS, scalar2=B0, op0=Alu.mult, op1=Alu.add,
                )
                # ACT: one fat exp fp8 -> f16 for the a-slots
                nc.scalar.activation(
                    out=eb[:, NB + NC_ : C, 0:F], in_=lb8[:, 0:NA, 0:F],
                    func=mybir.ActivationFunctionType.Exp,
                )
                # PE: 19 accumulating identity matmuls = the class sum.
                # Order by producer readiness: b (DMA+DVE), a (ACT), c
                # (GPSIMD+DVE).
                ps = ps_pool.tile([P, FMAX], fp32, tag="ps")
                order = (
                    list(range(0, NB))
                    + list(range(NB + NC_, C))
                    + list(range(NB, NB + NC_))
                )
                for i, k in enumerate(order):
                    nc.tensor.matmul(
                        out=ps[:, 0:F], lhsT=id_t[:], rhs=eb[:, k, 0:F],
                        start=(i == 0), stop=(i == C - 1),
                    )

                pending.append((j, F, lbb, ps))
                if len(pending) > LAG:
                    emit_tail(*pending.pop(0))
                off += F

            for args in pending:
                emit_tail(*args)

            nc.sync.dma_start(out=acc[:, :], in_=acc_t[:])
    nc.finalize()
    return nc


def _host_fallback(seg_logit, seg_label):
    """Exact numpy replication of the reference (quantile path included)."""
    x = np.asarray(seg_logit, dtype=np.float32)
    lbl = np.asarray(seg_label)
    Bn, Cn = x.shape[0], x.shape[1]
    xf = x.reshape(Bn, Cn, -1)
    m = xf.max(axis=1, keepdims=True)
    e = np.exp(xf - m)
    lse = np.log(e.sum(axis=1, keepdims=True)) + m
    logp = xf - lse
    l2 = np.where(lbl == IGNORE_INDEX, 0, lbl).reshape(Bn, 1, -1).astype(np.int64)
    lp_at = np.take_along_axis(logp, l2, axis=1)[:, 0]
    prob = np.exp(lp_at)
    sortp = np.sort(prob.reshape(-1))
    idx = min(MIN_KEPT * Bn, sortp.shape[0] - 1)
    thr = max(float(sortp[idx]), np.float32(0.7))
    wgt = (prob < thr).astype(np.float32)
    return np.float32((-lp_at * wgt).astype(np.float64).mean())


def _prep_core(x, lbl):
    """Per-pixel class transposition (slot0 <-> label), clamp, stage the
    b-slots as f16 and the a/c-slots as fp8-e3m4, chunk-blocked."""
    xp = np.empty((C, HW), dtype=np.float32)
    xp[0] = np.take_along_axis(x, lbl[None, :], axis=0)[0]
    for k in range(1, C):
        xp[k] = np.where(lbl == k, x[0], x[k])
    np.clip(xp, -10.0, 10.0, out=xp)

    def blocked(rows, dtype):
        t = rows.reshape(-1, P, FREE).transpose(1, 0, 2)  # [P, n, FREE]
        n = t.shape[1]
        blocks = []
        o = 0
        for f in CHUNKS:
            blocks.append(np.ascontiguousarray(t[:, :, o : o + f]).reshape(P, n * f))
            o += f
        return np.ascontiguousarray(np.concatenate(blocks, axis=1)).astype(dtype)

    xb = blocked(xp[0:NB], np.float16)
    # lb8 = [a-slots | c-slots]
    x8 = blocked(np.concatenate([xp[NB + NC_ : C], xp[NB : NB + NC_]], axis=0),
                 ml_dtypes.float8_e3m4)
    return xb, x8


def kernel(seg_logit, seg_label):
    from concourse import bass_utils

    x = np.ascontiguousarray(np.asarray(seg_logit, dtype=np.float32)).reshape(
        B, C, HW
    )
    lbl = np.asarray(seg_label)
    lbl = np.where(lbl == IGNORE_INDEX, 0, lbl).astype(np.int64).reshape(B, HW)

    idm = np.eye(P, dtype=np.float16)
    in_maps = []
    for b in range(B):
        xb, x8 = _prep_core(x[b], lbl[b])
        in_maps.append({"logit8": x8, "logitb": xb, "ident": idm})

    if "nc" not in _CACHE:
        _CACHE["nc"] = _build_nc()
    nc = _CACHE["nc"]

    res = bass_utils.run_bass_kernel_spmd(nc, in_maps, core_ids=list(range(B)))

    racc = 0.0
    wacc = 0.0
    for r in res.results:
        a = r["acc"]
        racc += float(a[:, :NCHUNK].sum(dtype=np.float64))
        wacc += float(a[:, NCHUNK:].sum(dtype=np.float64))

    if wacc <= MIN_KEPT * B:
        # quantile threshold exceeds 0.7 -> exact host path (rare/never for
        # the target distribution)
        return _host_fallback(seg_logit, seg_label)

    # racc sums max(w, M0) = -min(v, C0) - CLN per pixel; undo the constants
    total = racc + CLN * N_TOTAL + C0 * N_TOTAL - C0 * wacc
    return np.float32(total / N_TOTAL)


# revision 5
# speedup vs baseline: 1.5244x; 1.5244x over previous
"""OHEM cross-entropy loss kernel for Trainium2 (8 NeuronCores, Bass/Tile).

Math (matches reference.py):
    logp   = log_softmax(seg_logit, axis=1)          # [B,C,H,W], C=19
    x_l    = logp at label (ignore 255 -> class 0)
    prob   = exp(x_l)
    thr    = max(sort(prob.flatten())[MIN_KEPT*B], 0.7)
    loss   = mean(-x_l * (prob < thr))

Device strategy (data-parallel over B across 8 cores, one image per core).
The loss is a global mean over pixels (pixel order irrelevant) and the 2e-2
harness tolerance admits aggressive per-term approximation as long as the
per-pixel errors are mean-zero (4.2M pixels average them away; validated
~1.5e-4 end to end on the target distribution). Per pixel the device needs
x_l and lse = ln(sum_c exp(x_c)). HW-measured rates drive the split
(DVE tensor_scalar/tensor_copy reach their fast mode ONLY on flat
single-dim APs; accum_out or PSUM operands drop them to 1x; ACT is
0.833 ns/elem for any function; PE is clock-capped ~1.2 GHz here):

  - Host applies a PER-PIXEL class transposition (slot0 <-> label), so the
    label logit sits at class-slot 0 and the 19-way gather disappears
    (sumexp is permutation-invariant). ALL 19 slots staged fp8-e3m4,
    chunk-blocked -> one flat contiguous DMA per chunk, 19 B/pixel.
  - DVE: Schraudolph bit-trick exp for 12 slots in ONE flat 4x
    tensor_scalar (fp8 in, int16 out, ~0.26 ns/elem):
    bits16(e^x) ~= round(x*1477.32 + 15301.3); the -58.7 zeroes the mean
    chord error of 2^f. Then an in-place pairwise tensor_tensor tree (2x)
    folds 9 of those slots to one partial.
  - ACT: native exp for the other 7 slots (one flat instruction/chunk).
  - PE: 11 accumulating identity matmuls per chunk gather everything
    into PSUM [128,F] f32: 3 DVE slots + 7 ACT slots + the tree partial.
  - lse via a second Schraudolph on the PSUM bits: one DVE
    scalar_tensor_tensor w = bits32(sumexp)*ln2/2^23 - x_l (x_l read
    straight from the fp8 slot-0 plane; w ~ 88+lse-x_l, its f16
    rounding is mean-zero dither).
  - ACT Relu / Sign with a per-partition f32 bias of -(127*ln2 - CLN - C0)
    and accum_out reduce w over 3-chunk groups into the loss partials:
    relu_acc = sum(max(v-C0-.., 0)-ish), sign_acc -> kept-count, where
    CLN=0.0397 zeroes the ln-side mantissa-chord mean.

    Per-chunk tails are issued TWO CHUNKS LATE so the in-order DVE queue
    never waits on the PE accumulation of the current chunk.

    Host combines partials: loss = (relu_sum - C0*wacc)/N, falling back
    to an exact host path if wacc <= MIN_KEPT*B (never for the target
    distribution).
"""

import numpy as np
import ml_dtypes

B = 8
C = 19
H, W = 512, 1024
HW = H * W            # 524288 pixels per image/core
P = 128               # SBUF partitions
FREE = HW // P        # 4096 pixels per partition
# head/tail chunks smaller: cuts pipeline-fill and drain latency.
# F <= 512 (one PSUM bank / max moving free dim per matmul).
CHUNKS = [256, 512, 512, 512, 512, 512, 512, 512, 256]
assert sum(CHUNKS) == FREE
FMAX = max(CHUNKS)
NCHUNK = len(CHUNKS)
GROUP = 3             # chunks per accumulation group
NGRP = (NCHUNK + GROUP - 1) // GROUP

ND = 12               # DVE Schraudolph slots (slot 0 = label logit)
NPE_D = 3             # of those, fed to PE directly (slots 0..2)
NTREE = ND - NPE_D    # summed by the DVE tensor_tensor tree (slots 3..11)
NA = C - ND           # 7 ACT native-exp slots (12..18)

C0 = float(np.log(np.float32(0.7)))
AS = 1477.3196        # 1024*log2(e)
B0 = 15360.0 - 58.7   # f16 exponent bias 15<<10, minus mean chord error
K1 = float(np.log(2.0) / 2**23)
CLN = 0.0397          # mean of ln(1+f) - f*ln2 over the mantissa chord
TH = float(127 * np.log(2.0) - CLN - C0)   # w threshold (f32 bias = -TH)
MIN_KEPT = 100000
IGNORE_INDEX = 255
N_TOTAL = B * HW
LAG = 2               # chunks between PSUM production and its DVE tail

_CACHE = {}


def _build_nc():
    import concourse.bacc as bacc
    import concourse.mybir as mybir
    import concourse.tile as tile

    fp16 = mybir.dt.float16
    fp32 = mybir.dt.float32
    fp8 = mybir.dt.float8e3
    i16 = mybir.dt.int16
    i32 = mybir.dt.int32
    Alu = mybir.AluOpType
    Act = mybir.ActivationFunctionType

    nc = bacc.Bacc()
    # chunk-blocked flat layout: per partition, chunk j's 19*F_j fp8 block
    # is contiguous (slot-major) -> one large contiguous DMA per chunk
    logit8 = nc.dram_tensor("logit8", [P, C * FREE], fp8, kind="ExternalInput")
    ident = nc.dram_tensor("ident", [P, P], fp16, kind="ExternalInput")
    acc = nc.dram_tensor("acc", [P, 2 * NGRP], fp32, kind="ExternalOutput")

    # group offset of each chunk's w segment, and group boundaries
    goff = []
    gend = []
    o = 0
    for j, F in enumerate(CHUNKS):
        if j % GROUP == 0:
            o = 0
        goff.append(o)
        o += F
        if j % GROUP == GROUP - 1 or j == NCHUNK - 1:
            gend.append(o)

    with tile.TileContext(nc) as tc:
        with (
            tc.tile_pool(name="lb8", bufs=3) as lb8_pool,
            tc.tile_pool(name="eb", bufs=3) as eb_pool,
            tc.tile_pool(name="ps", bufs=3, space="PSUM") as ps_pool,
            tc.tile_pool(name="wb", bufs=2) as wb_pool,
            tc.tile_pool(name="scr", bufs=2) as scr_pool,
            tc.tile_pool(name="one", bufs=1) as one_pool,
        ):
            acc_t = one_pool.tile([P, 2 * NGRP], fp32)
            id_t = one_pool.tile([P, P], fp16)
            bias_t = one_pool.tile([P, 1], fp32)
            nc.gpsimd.memset(bias_t[:], -TH)

            wbufs = {}   # group -> tile

            def emit_tail(j, F, lb8, ps):
                # w = bits32(sumexp)*K1 - x_l  (STT, 1x: PSUM operand)
                g = j // GROUP
                if g not in wbufs:
                    wbt = wb_pool.tile([P, GROUP * FMAX], fp16, tag="wb")
                    wbufs[g] = wbt
                wb = wbufs[g]
                o = goff[j]
                nc.vector.scalar_tensor_tensor(
                    out=wb[:, o : o + F], in0=ps[:, 0:F].bitcast(i32),
                    scalar=K1, in1=lb8[:, 0:F], op0=Alu.mult,
                    op1=Alu.subtract,
                )
                if j % GROUP == GROUP - 1 or j == NCHUNK - 1:
                    # group accumulations on ACT (dtype-independent 1x;
                    # Relu/Sign/Exp share one table set)
                    E = gend[g]
                    scr = scr_pool.tile([P, GROUP * FMAX], fp16, tag="scr")
                    nc.scalar.activation(
                        out=scr[:, 0:E], in_=wb[:, 0:E], func=Act.Relu,
                        bias=bias_t[:], scale=1.0,
                        accum_out=acc_t[:, g : g + 1],
                    )
                    scr2 = scr_pool.tile([P, GROUP * FMAX], fp16, tag="scr2")
                    nc.scalar.activation(
                        out=scr2[:, 0:E], in_=wb[:, 0:E], func=Act.Sign,
                        bias=bias_t[:], scale=1.0,
                        accum_out=acc_t[:, NGRP + g : NGRP + g + 1],
                    )

            pending = []  # (j, F, lb8, ps) awaiting their reduction tail
            off = 0
            for j, F in enumerate(CHUNKS):
                lb8 = lb8_pool.tile([P, C * FMAX], fp8, tag="lb8")
                nc.sync.dma_start(
                    out=lb8[:, 0 : C * F],
                    in_=logit8[:, C * off : C * (off + F)],
                )
                if j == 0:
                    nc.sync.dma_start(out=id_t[:], in_=ident[:, :])

                eb = eb_pool.tile([P, C * FMAX], fp16, tag="eb")
                # DVE: one flat 4x Schraudolph for the 12 DVE slots
                nc.vector.tensor_scalar(
                    out=eb[:, 0 : ND * F].bitcast(i16),
                    in0=lb8[:, 0 : ND * F],
                    scalar1=AS, scalar2=B0, op0=Alu.mult, op1=Alu.add,
                )
                # ACT: one flat exp for the 7 ACT slots
                nc.scalar.activation(
                    out=eb[:, ND * F : C * F], in_=lb8[:, ND * F : C * F],
                    func=Act.Exp,
                )
                # DVE: in-place pairwise tree over slots [NPE_D, ND)
                lo, n = NPE_D, NTREE
                while n > 1:
                    h = n // 2
                    nc.vector.tensor_tensor(
                        out=eb[:, lo * F : (lo + h) * F],
                        in0=eb[:, lo * F : (lo + h) * F],
                        in1=eb[:, (lo + n - h) * F : (lo + n) * F],
                        op=Alu.add,
                    )
                    n -= h
                # PE: accumulating identity matmuls -> PSUM [128,F] f32.
                # Order by producer readiness: direct DVE slots, ACT slots,
                # tree root last.
                ps = ps_pool.tile([P, FMAX], fp32, tag="ps")
                slots = list(range(NPE_D)) + list(range(ND, C)) + [NPE_D]
                for i, k in enumerate(slots):
                    nc.tensor.matmul(
                        out=ps[:, 0:F], lhsT=id_t[:],
                        rhs=eb[:, k * F : (k + 1) * F],
                        start=(i == 0), stop=(i == len(slots) - 1),
                    )

                pending.append((j, F, lb8, ps))
                if len(pending) > LAG:
                    emit_tail(*pending.pop(0))
                off += F

            for args in pending:
                emit_tail(*args)

            nc.sync.dma_start(out=acc[:, :], in_=acc_t[:])
    nc.finalize()
    return nc


def _host_fallback(seg_logit, seg_label):
    """Exact numpy replication of the reference (quantile path included)."""
    x = np.asarray(seg_logit, dtype=np.float32)
    lbl = np.asarray(seg_label)
    Bn, Cn = x.shape[0], x.shape[1]
    xf = x.reshape(Bn, Cn, -1)
    m = xf.max(axis=1, keepdims=True)
    e = np.exp(xf - m)
    lse = np.log(e.sum(axis=1, keepdims=True)) + m
    logp = xf - lse
    l2 = np.where(lbl == IGNORE_INDEX, 0, lbl).reshape(Bn, 1, -1).astype(np.int64)
    lp_at = np.take_along_axis(logp, l2, axis=1)[:, 0]
    prob = np.exp(lp_at)
    sortp = np.sort(prob.reshape(-1))
    idx = min(MIN_KEPT * Bn, sortp.shape[0] - 1)
    thr = max(float(sortp[idx]), np.float32(0.7))
    wgt = (prob < thr).astype(np.float32)
    return np.float32((-lp_at * wgt).astype(np.float64).mean())


def _prep_core(x, lbl):
    """Per-pixel class transposition (slot0 <-> label), clamp, stage all
    slots as fp8-e3m4, chunk-blocked flat (slot-major inside each chunk)."""
    xp = np.empty((C, HW), dtype=np.float32)
    xp[0] = np.take_along_axis(x, lbl[None, :], axis=0)[0]
    for k in range(1, C):
        xp[k] = np.where(lbl == k, x[0], x[k])
    np.clip(xp, -10.0, 10.0, out=xp)

    t = xp.reshape(C, P, FREE).transpose(1, 0, 2)   # [P, C, FREE]
    blocks = []
    o = 0
    for f in CHUNKS:
        blocks.append(np.ascontiguousarray(t[:, :, o : o + f]).reshape(P, C * f))
        o += f
    return np.ascontiguousarray(np.concatenate(blocks, axis=1)).astype(
        ml_dtypes.float8_e3m4
    )


def kernel(seg_logit, seg_label):
    from concourse import bass_utils

    x = np.ascontiguousarray(np.asarray(seg_logit, dtype=np.float32)).reshape(
        B, C, HW
    )
    lbl = np.asarray(seg_label)
    lbl = np.where(lbl == IGNORE_INDEX, 0, lbl).astype(np.int64).reshape(B, HW)

    idm = np.eye(P, dtype=np.float16)
    in_maps = []
    for b in range(B):
        in_maps.append({"logit8": _prep_core(x[b], lbl[b]), "ident": idm})

    if "nc" not in _CACHE:
        _CACHE["nc"] = _build_nc()
    nc = _CACHE["nc"]

    res = bass_utils.run_bass_kernel_spmd(nc, in_maps, core_ids=list(range(B)))

    relu_sum = 0.0
    sign_sum = 0.0
    for r in res.results:
        a = r["acc"]
        relu_sum += float(a[:, :NGRP].sum(dtype=np.float64))
        sign_sum += float(a[:, NGRP:].sum(dtype=np.float64))

    wacc = (sign_sum + N_TOTAL) / 2.0
    if wacc <= MIN_KEPT * B:
        # quantile threshold exceeds 0.7 -> exact host path (rare/never for
        # the target distribution)
        return _host_fallback(seg_logit, seg_label)

    return np.float32((relu_sum - C0 * wacc) / N_TOTAL)


# revision 6
# speedup vs baseline: 1.6705x; 1.0958x over previous
"""OHEM cross-entropy loss kernel for Trainium2 (8 NeuronCores, Bass/Tile).

Math (matches reference.py):
    logp   = log_softmax(seg_logit, axis=1)          # [B,C,H,W], C=19
    x_l    = logp at label (ignore 255 -> class 0)
    prob   = exp(x_l)
    thr    = max(sort(prob.flatten())[MIN_KEPT*B], 0.7)
    loss   = mean(-x_l * (prob < thr))

Device strategy (data-parallel over B across 8 cores, one image per core).
The loss is a global mean over pixels (pixel order irrelevant) and the 2e-2
harness tolerance admits aggressive per-term approximation as long as the
per-pixel errors are mean-zero (4.2M pixels average them away; validated
~1.5e-4 end to end on the target distribution). Per pixel the device needs
x_l and lse = ln(sum_c exp(x_c)). HW-measured rates drive the split
(DVE tensor_scalar/tensor_copy reach their fast mode ONLY on flat
single-dim APs; accum_out or PSUM operands drop them to 1x; ACT is
0.833 ns/elem for any function; PE is clock-capped ~1.2 GHz here):

  - Host applies a PER-PIXEL class transposition (slot0 <-> label), so the
    label logit sits at class-slot 0 and the 19-way gather disappears
    (sumexp is permutation-invariant). ALL 19 slots staged fp8-e3m4,
    chunk-blocked -> one flat contiguous DMA per chunk, 19 B/pixel.
  - DVE: Schraudolph bit-trick exp for 12 slots in ONE flat 4x
    tensor_scalar (fp8 in, int16 out, ~0.26 ns/elem):
    bits16(e^x) ~= round(x*1477.32 + 15301.3); the -58.7 zeroes the mean
    chord error of 2^f. Then an in-place pairwise tensor_tensor tree (2x)
    folds 9 of those slots to one partial.
  - ACT: native exp for the other 7 slots (one flat instruction/chunk).
  - PE: 11 accumulating identity matmuls per chunk gather everything
    into PSUM [128,F] f32: 3 DVE slots + 7 ACT slots + the tree partial.
  - lse via a second Schraudolph on the PSUM bits: one DVE
    scalar_tensor_tensor w = bits32(sumexp)*ln2/2^23 - x_l (x_l read
    straight from the fp8 slot-0 plane; w ~ 88+lse-x_l, its f16
    rounding is mean-zero dither).
  - ACT Relu / Sign with a per-partition f32 bias of -(127*ln2 - CLN - C0)
    and accum_out reduce w over 3-chunk groups into the loss partials:
    relu_acc = sum(max(v-C0-.., 0)-ish), sign_acc -> kept-count, where
    CLN=0.0397 zeroes the ln-side mantissa-chord mean.

    Per-chunk tails are issued TWO CHUNKS LATE so the in-order DVE queue
    never waits on the PE accumulation of the current chunk.

    Host combines partials: loss = (relu_sum - C0*wacc)/N, falling back
    to an exact host path if wacc <= MIN_KEPT*B (never for the target
    distribution).
"""

import numpy as np
import ml_dtypes

B = 8
C = 19
H, W = 512, 1024
HW = H * W            # 524288 pixels per image/core
P = 128               # SBUF partitions
FREE = HW // P        # 4096 pixels per partition
# head/tail chunks smaller: cuts pipeline-fill and drain latency.
# F <= 512 (one PSUM bank / max moving free dim per matmul).
CHUNKS = [256, 512, 512, 512, 512, 512, 512, 512, 256]
assert sum(CHUNKS) == FREE
FMAX = max(CHUNKS)
NCHUNK = len(CHUNKS)
GROUP = 3             # chunks per accumulation group
NGRP = (NCHUNK + GROUP - 1) // GROUP

NF16 = 6              # f16-staged DVE slots (slot 0 = label logit): 4x schrau
ND8 = 6               # fp8-staged DVE slots (2x schrau)
ND = NF16 + ND8       # 12 DVE Schraudolph slots
NA = C - ND           # 7 ACT native-exp slots (12..18)
# class sum: slots 1..11 via the DVE tensor_tensor tree (11 leaves),
# slot 0 + the 7 ACT slots + the tree root via 9 PE identity matmuls

C0 = float(np.log(np.float32(0.7)))
AS = 1477.3196        # 1024*log2(e)
B0 = 15360.0 - 58.7   # f16 exponent bias 15<<10, minus mean chord error
K1 = float(np.log(2.0) / 2**23)
CLN = 0.0397          # mean of ln(1+f) - f*ln2 over the mantissa chord
TH = float(127 * np.log(2.0) - CLN - C0)   # w threshold (f32 bias = -TH)
MIN_KEPT = 100000
IGNORE_INDEX = 255
N_TOTAL = B * HW
LAG = 2               # chunks between PSUM production and its DVE tail

_CACHE = {}


def _build_nc():
    import concourse.bacc as bacc
    import concourse.mybir as mybir
    import concourse.tile as tile

    fp16 = mybir.dt.float16
    fp32 = mybir.dt.float32
    fp8 = mybir.dt.float8e3
    i16 = mybir.dt.int16
    i32 = mybir.dt.int32
    Alu = mybir.AluOpType
    Act = mybir.ActivationFunctionType

    nc = bacc.Bacc()
    # chunk-blocked flat layouts: per partition, chunk j's slot-major block
    # is contiguous -> one large contiguous DMA per chunk per tensor
    logitb = nc.dram_tensor("logitb", [P, NF16 * FREE], fp16,
                            kind="ExternalInput")
    logit8 = nc.dram_tensor("logit8", [P, (ND8 + NA) * FREE], fp8,
                            kind="ExternalInput")
    ident = nc.dram_tensor("ident", [P, P], fp16, kind="ExternalInput")
    acc = nc.dram_tensor("acc", [P, 2 * NGRP], fp32, kind="ExternalOutput")

    # group offset of each chunk's w segment, and group boundaries
    goff = []
    gend = []
    o = 0
    for j, F in enumerate(CHUNKS):
        if j % GROUP == 0:
            o = 0
        goff.append(o)
        o += F
        if j % GROUP == GROUP - 1 or j == NCHUNK - 1:
            gend.append(o)

    with tile.TileContext(nc) as tc:
        with (
            tc.tile_pool(name="lbb", bufs=4) as lbb_pool,
            tc.tile_pool(name="lb8", bufs=3) as lb8_pool,
            tc.tile_pool(name="eb", bufs=3) as eb_pool,
            tc.tile_pool(name="ps", bufs=3, space="PSUM") as ps_pool,
            tc.tile_pool(name="wb", bufs=2) as wb_pool,
            tc.tile_pool(name="scr", bufs=2) as scr_pool,
            tc.tile_pool(name="one", bufs=1) as one_pool,
        ):
            acc_t = one_pool.tile([P, 2 * NGRP], fp32)
            id_t = one_pool.tile([P, P], fp16)
            bias_t = one_pool.tile([P, 1], fp32)
            nc.gpsimd.memset(bias_t[:], -TH)

            wbufs = {}   # group -> tile

            def emit_tail(j, F, lbb, ps):
                # w = bits32(sumexp)*K1 - x_l  (STT, 1x: PSUM operand)
                g = j // GROUP
                if g not in wbufs:
                    wbt = wb_pool.tile([P, GROUP * FMAX], fp16, tag="wb")
                    wbufs[g] = wbt
                wb = wbufs[g]
                o = goff[j]
                nc.vector.scalar_tensor_tensor(
                    out=wb[:, o : o + F], in0=ps[:, 0:F].bitcast(i32),
                    scalar=K1, in1=lbb[:, 0:F], op0=Alu.mult,
                    op1=Alu.subtract,
                )
                if j % GROUP == GROUP - 1 or j == NCHUNK - 1:
                    # group accumulations on ACT (dtype-independent 1x;
                    # Relu/Sign/Exp share one table set)
                    E = gend[g]
                    scr = scr_pool.tile([P, GROUP * FMAX], fp16, tag="scr")
                    nc.scalar.activation(
                        out=scr[:, 0:E], in_=wb[:, 0:E], func=Act.Relu,
                        bias=bias_t[:], scale=1.0,
                        accum_out=acc_t[:, g : g + 1],
                    )
                    scr2 = scr_pool.tile([P, GROUP * FMAX], fp16, tag="scr2")
                    nc.scalar.activation(
                        out=scr2[:, 0:E], in_=wb[:, 0:E], func=Act.Sign,
                        bias=bias_t[:], scale=1.0,
                        accum_out=acc_t[:, NGRP + g : NGRP + g + 1],
                    )

            pending = []  # (j, F, lbb, ps) awaiting their reduction tail
            off = 0
            N8 = ND8 + NA
            for j, F in enumerate(CHUNKS):
                lbb = lbb_pool.tile([P, NF16 * FMAX], fp16, tag="lbb")
                nc.sync.dma_start(
                    out=lbb[:, 0 : NF16 * F],
                    in_=logitb[:, NF16 * off : NF16 * (off + F)],
                )
                lb8 = lb8_pool.tile([P, N8 * FMAX], fp8, tag="lb8")
                nc.sync.dma_start(
                    out=lb8[:, 0 : N8 * F],
                    in_=logit8[:, N8 * off : N8 * (off + F)],
                )
                if j == 0:
                    nc.sync.dma_start(out=id_t[:], in_=ident[:, :])

                eb = eb_pool.tile([P, C * FMAX], fp16, tag="eb")
                # DVE: flat Schraudolphs -- 4x for the f16 slots, 2x fp8
                nc.vector.tensor_scalar(
                    out=eb[:, 0 : NF16 * F].bitcast(i16),
                    in0=lbb[:, 0 : NF16 * F],
                    scalar1=AS, scalar2=B0, op0=Alu.mult, op1=Alu.add,
                )
                nc.vector.tensor_scalar(
                    out=eb[:, NF16 * F : ND * F].bitcast(i16),
                    in0=lb8[:, 0 : ND8 * F],
                    scalar1=AS, scalar2=B0, op0=Alu.mult, op1=Alu.add,
                )
                # ACT: one flat exp for the 7 ACT slots
                nc.scalar.activation(
                    out=eb[:, ND * F : C * F], in_=lb8[:, ND8 * F : N8 * F],
                    func=Act.Exp,
                )
                # DVE: in-place pairwise tree over slots [1, ND)
                lo, n = 1, ND - 1
                while n > 1:
                    h = n // 2
                    nc.vector.tensor_tensor(
                        out=eb[:, lo * F : (lo + h) * F],
                        in0=eb[:, lo * F : (lo + h) * F],
                        in1=eb[:, (lo + n - h) * F : (lo + n) * F],
                        op=Alu.add,
                    )
                    n -= h
                # PE: accumulating identity matmuls -> PSUM [128,F] f32.
                # Order by producer readiness: slot 0, ACT slots, tree root.
                ps = ps_pool.tile([P, FMAX], fp32, tag="ps")
                slots = [0] + list(range(ND, C)) + [1]
                for i, k in enumerate(slots):
                    nc.tensor.matmul(
                        out=ps[:, 0:F], lhsT=id_t[:],
                        rhs=eb[:, k * F : (k + 1) * F],
                        start=(i == 0), stop=(i == len(slots) - 1),
                    )

                pending.append((j, F, lbb, ps))
                if len(pending) > LAG:
                    emit_tail(*pending.pop(0))
                off += F

            for args in pending:
                emit_tail(*args)

            nc.sync.dma_start(out=acc[:, :], in_=acc_t[:])
    nc.finalize()
    return nc


def _host_fallback(seg_logit, seg_label):
    """Exact numpy replication of the reference (quantile path included)."""
    x = np.asarray(seg_logit, dtype=np.float32)
    lbl = np.asarray(seg_label)
    Bn, Cn = x.shape[0], x.shape[1]
    xf = x.reshape(Bn, Cn, -1)
    m = xf.max(axis=1, keepdims=True)
    e = np.exp(xf - m)
    lse = np.log(e.sum(axis=1, keepdims=True)) + m
    logp = xf - lse
    l2 = np.where(lbl == IGNORE_INDEX, 0, lbl).reshape(Bn, 1, -1).astype(np.int64)
    lp_at = np.take_along_axis(logp, l2, axis=1)[:, 0]
    prob = np.exp(lp_at)
    sortp = np.sort(prob.reshape(-1))
    idx = min(MIN_KEPT * Bn, sortp.shape[0] - 1)
    thr = max(float(sortp[idx]), np.float32(0.7))
    wgt = (prob < thr).astype(np.float32)
    return np.float32((-lp_at * wgt).astype(np.float64).mean())


def _blocked(rows, dtype):
    n = rows.shape[0]
    t = rows.reshape(n, P, FREE).transpose(1, 0, 2)   # [P, n, FREE]
    blocks = []
    o = 0
    for f in CHUNKS:
        blocks.append(np.ascontiguousarray(t[:, :, o : o + f]).reshape(P, n * f))
        o += f
    return np.ascontiguousarray(np.concatenate(blocks, axis=1)).astype(dtype)


def _prep_core(x, lbl):
    """Per-pixel class transposition (slot0 <-> label), clamp, stage the
    first NF16 slots f16 and the rest fp8-e3m4, chunk-blocked flat."""
    xp = np.empty((C, HW), dtype=np.float32)
    xp[0] = np.take_along_axis(x, lbl[None, :], axis=0)[0]
    for k in range(1, C):
        xp[k] = np.where(lbl == k, x[0], x[k])
    np.clip(xp, -10.0, 10.0, out=xp)
    return (_blocked(xp[0:NF16], np.float16),
            _blocked(xp[NF16:C], ml_dtypes.float8_e3m4))


def kernel(seg_logit, seg_label):
    from concourse import bass_utils

    x = np.ascontiguousarray(np.asarray(seg_logit, dtype=np.float32)).reshape(
        B, C, HW
    )
    lbl = np.asarray(seg_label)
    lbl = np.where(lbl == IGNORE_INDEX, 0, lbl).astype(np.int64).reshape(B, HW)

    idm = np.eye(P, dtype=np.float16)
    in_maps = []
    for b in range(B):
        xb, x8 = _prep_core(x[b], lbl[b])
        in_maps.append({"logitb": xb, "logit8": x8, "ident": idm})

    if "nc" not in _CACHE:
        _CACHE["nc"] = _build_nc()
    nc = _CACHE["nc"]

    res = bass_utils.run_bass_kernel_spmd(nc, in_maps, core_ids=list(range(B)))

    relu_sum = 0.0
    sign_sum = 0.0
    for r in res.results:
        a = r["acc"]
        relu_sum += float(a[:, :NGRP].sum(dtype=np.float64))
        sign_sum += float(a[:, NGRP:].sum(dtype=np.float64))

    wacc = (sign_sum + N_TOTAL) / 2.0
    if wacc <= MIN_KEPT * B:
        # quantile threshold exceeds 0.7 -> exact host path (rare/never for
        # the target distribution)
        return _host_fallback(seg_logit, seg_label)

    return np.float32((relu_sum - C0 * wacc) / N_TOTAL)


# revision 7
# speedup vs baseline: 2.0439x; 1.2235x over previous
"""OHEM cross-entropy loss kernel for Trainium2 (8 NeuronCores, Bass/Tile).

Math (matches reference.py):
    logp   = log_softmax(seg_logit, axis=1)          # [B,C,H,W], C=19
    x_l    = logp at label (ignore 255 -> class 0)
    prob   = exp(x_l)
    thr    = max(sort(prob.flatten())[MIN_KEPT*B], 0.7)
    loss   = mean(-x_l * (prob < thr))

Device strategy (data-parallel over B across 8 cores, one image per core).
The loss is a global mean over pixels (pixel order irrelevant) and the 2e-2
harness tolerance admits aggressive per-term approximation as long as the
per-pixel errors are mean-zero (4.2M pixels average them away; validated
~5e-4 end to end on the target distribution). Per pixel the device needs
x_l and lse = ln(sum_c exp(x_c)). HW-measured rates drive the design
(DVE fast modes need flat single-dim APs and die on accum_out/PSUM
operands; ACT is 0.833 ns/elem for any function; PE is ~1.2 GHz here and
fp8 DoubleRow sums TWO planes per matmul at ~0.83 ns/column):

  - Host applies a PER-PIXEL class transposition (slot0 <-> label), so the
    label logit sits at class-slot 0 and the 19-way gather disappears
    (sumexp is permutation-invariant). ALL 19 slots staged fp8-e3m4
    chunk-blocked -> ONE flat contiguous DMA per chunk, 19 B/pixel.
  - exp lands as fp8-e4m3 *bit patterns* (e^x quantized ~9%/step,
    mean-zero):
      DVE: 12 slots via the Schraudolph bit trick in ONE flat
      tensor_scalar (fp8 in, int8 out): bits8(e^x) ~= round(x*11.5416 +
      55.54); the -0.46 in B8 zeroes the mean 2^f chord error.
      ACT: native exp for 7 slots, fp8e4 output rounding (one flat
      instruction per chunk).
  - PE: the whole 19->1 class sum in 10 accumulating matmuls per chunk:
    9 fp8 DoubleRow pair-matmuls (lhsT = [I|I], rhs = two adjacent class
    planes as a 3D [128,2,F] AP) + 1 plain fp8 matmul for plane 18,
    into PSUM [128,F] f32. No DVE tree at all.
  - lse via a second Schraudolph on the PSUM bits: one DVE
    scalar_tensor_tensor w = bits32(sumexp)*ln2/2^23 - x_l (x_l is an
    exact fp8->f16 cast of the slot-0 plane; w ~ 88+lse-x_l, its f16
    rounding is mean-zero dither).
  - ACT Relu / Sign with a per-partition f32 bias of -(127*ln2 - CLN - C0)
    and accum_out reduce w over 3-chunk groups into the loss partials
    (relu_acc -> kept-loss sum, sign_acc -> kept count); CLN=0.0397
    zeroes the ln-side mantissa-chord mean.

    Per-chunk tails are issued TWO CHUNKS LATE so the in-order DVE queue
    never waits on the PE accumulation of the current chunk.

    Host combines partials: loss = (relu_sum - C0*wacc)/N with
    wacc = (sign_sum + N)/2, falling back to an exact host path if
    wacc <= MIN_KEPT*B (never for the target distribution).
"""

import numpy as np
import ml_dtypes

B = 8
C = 19
H, W = 512, 1024
HW = H * W            # 524288 pixels per image/core
P = 128               # SBUF partitions
FREE = HW // P        # 4096 pixels per partition
# small head chunks cut pipeline-fill latency.
# F <= 512 (one PSUM bank / max moving free dim per matmul).
CHUNKS = [128, 384, 512, 512, 512, 512, 512, 512, 512]
assert sum(CHUNKS) == FREE
FMAX = max(CHUNKS)
NCHUNK = len(CHUNKS)
GROUP = 3             # chunks per accumulation group
NGRP = (NCHUNK + GROUP - 1) // GROUP

ND = 12               # DVE Schraudolph slots (slot 0 = label logit)
NA = C - ND           # 7 ACT native-exp slots (12..18)

C0 = float(np.log(np.float32(0.7)))
A8 = float(8 * np.log2(np.e))   # 11.5416: e4m3 bits per e-fold
B8 = 56.0 - 0.458               # e4m3 exponent bias 7<<3, minus chord mean
K1 = float(np.log(2.0) / 2**23)
CLN = 0.0397          # mean of ln(1+f) - f*ln2 over the mantissa chord
TH = float(127 * np.log(2.0) - CLN - C0)   # w threshold (f32 bias = -TH)
MIN_KEPT = 100000
IGNORE_INDEX = 255
N_TOTAL = B * HW
LAG = 2               # chunks between PSUM production and its DVE tail

_CACHE = {}


def _build_nc():
    import concourse.bacc as bacc
    import concourse.mybir as mybir
    import concourse.tile as tile

    fp16 = mybir.dt.float16
    fp32 = mybir.dt.float32
    fp8e3 = mybir.dt.float8e3
    fp8e4 = mybir.dt.float8e4
    i8 = mybir.dt.int8
    i32 = mybir.dt.int32
    Alu = mybir.AluOpType
    Act = mybir.ActivationFunctionType
    PM = mybir.MatmulPerfMode

    nc = bacc.Bacc()
    # chunk-blocked flat layout: per partition, chunk j's 19*F_j fp8 block
    # is contiguous (slot-major) -> one large contiguous DMA per chunk
    logit8 = nc.dram_tensor("logit8", [P, C * FREE], fp8e3,
                            kind="ExternalInput")
    ident2 = nc.dram_tensor("ident2", [P, 2 * P], fp8e4, kind="ExternalInput")
    acc = nc.dram_tensor("acc", [P, 2 * NGRP], fp32, kind="ExternalOutput")

    # group offset of each chunk's w segment, and group boundaries
    goff = []
    gend = []
    o = 0
    for j, F in enumerate(CHUNKS):
        if j % GROUP == 0:
            o = 0
        goff.append(o)
        o += F
        if j % GROUP == GROUP - 1 or j == NCHUNK - 1:
            gend.append(o)

    with tile.TileContext(nc) as tc:
        with (
            tc.tile_pool(name="lb8", bufs=3) as lb8_pool,
            tc.tile_pool(name="eb8", bufs=3) as eb8_pool,
            tc.tile_pool(name="xls", bufs=4) as xls_pool,
            tc.tile_pool(name="ps", bufs=3, space="PSUM") as ps_pool,
            tc.tile_pool(name="wb", bufs=2) as wb_pool,
            tc.tile_pool(name="scr", bufs=2) as scr_pool,
            tc.tile_pool(name="one", bufs=1) as one_pool,
        ):
            acc_t = one_pool.tile([P, 2 * NGRP], fp32)
            id2_t = one_pool.tile([P, 2, P], fp8e4)
            bias_t = one_pool.tile([P, 1], fp32)
            nc.gpsimd.memset(bias_t[:], -TH)

            wbufs = {}   # group -> tile

            def emit_tail(j, F, xls, ps):
                # w = bits32(sumexp)*K1 - x_l  (STT, 1x: PSUM operand)
                g = j // GROUP
                if g not in wbufs:
                    wbt = wb_pool.tile([P, GROUP * FMAX], fp16, tag="wb")
                    wbufs[g] = wbt
                wb = wbufs[g]
                o = goff[j]
                nc.vector.scalar_tensor_tensor(
                    out=wb[:, o : o + F], in0=ps[:, 0:F].bitcast(i32),
                    scalar=K1, in1=xls[:, 0:F], op0=Alu.mult,
                    op1=Alu.subtract,
                )
                if j % GROUP == GROUP - 1 or j == NCHUNK - 1:
                    # group accumulations on ACT (dtype-independent 1x;
                    # Relu/Sign/Exp share one table set)
                    E = gend[g]
                    scr = scr_pool.tile([P, GROUP * FMAX], fp16, tag="scr")
                    nc.scalar.activation(
                        out=scr[:, 0:E], in_=wb[:, 0:E], func=Act.Relu,
                        bias=bias_t[:], scale=1.0,
                        accum_out=acc_t[:, g : g + 1],
                    )
                    scr2 = scr_pool.tile([P, GROUP * FMAX], fp16, tag="scr2")
                    nc.scalar.activation(
                        out=scr2[:, 0:E], in_=wb[:, 0:E], func=Act.Sign,
                        bias=bias_t[:], scale=1.0,
                        accum_out=acc_t[:, NGRP + g : NGRP + g + 1],
                    )

            pending = []  # (j, F, xls, ps) awaiting their reduction tail
            off = 0
            for j, F in enumerate(CHUNKS):
                lb8 = lb8_pool.tile([P, C * FMAX], fp8e3, tag="lb8")
                nc.sync.dma_start(
                    out=lb8[:, 0 : C * F],
                    in_=logit8[:, C * off : C * (off + F)],
                )
                if j == 0:
                    nc.sync.dma_start(
                        out=id2_t[:],
                        in_=ident2[:, :].rearrange("p (c f) -> p c f", c=2),
                    )

                # DVE: exact cast of the label plane for the tail
                xls = xls_pool.tile([P, FMAX], fp16, tag="xls")
                nc.vector.tensor_scalar(
                    out=xls[:, 0:F], in0=lb8[:, 0:F],
                    scalar1=1.0, scalar2=0.0, op0=Alu.mult, op1=Alu.add,
                )
                eb8 = eb8_pool.tile([P, C * FMAX], fp8e4, tag="eb8")
                # DVE: one flat Schraudolph -> e4m3 bits for the 12 DVE slots
                nc.vector.tensor_scalar(
                    out=eb8[:, 0 : ND * F].bitcast(i8),
                    in0=lb8[:, 0 : ND * F],
                    scalar1=A8, scalar2=B8, op0=Alu.mult, op1=Alu.add,
                )
                # ACT: one flat exp -> fp8e4 for the 7 ACT slots
                nc.scalar.activation(
                    out=eb8[:, ND * F : C * F], in_=lb8[:, ND * F : C * F],
                    func=Act.Exp,
                )
                # PE: 9 DoubleRow pair-matmuls + 1 plain matmul = class sum
                ps = ps_pool.tile([P, FMAX], fp32, tag="ps")
                for i in range(9):
                    nc.tensor.matmul(
                        out=ps[:, 0:F], lhsT=id2_t[:],
                        rhs=eb8[:, 2 * i * F : (2 * i + 2) * F].rearrange(
                            "p (c f) -> p c f", c=2
                        ),
                        start=(i == 0), stop=False,
                        perf_mode=PM.DoubleRow,
                    )
                nc.tensor.matmul(
                    out=ps[:, 0:F], lhsT=id2_t[:, 0, :],
                    rhs=eb8[:, 18 * F : C * F],
                    start=False, stop=True,
                )

                pending.append((j, F, xls, ps))
                if len(pending) > LAG:
                    emit_tail(*pending.pop(0))
                off += F

            for args in pending:
                emit_tail(*args)

            nc.sync.dma_start(out=acc[:, :], in_=acc_t[:])
    nc.finalize()
    return nc


def _host_fallback(seg_logit, seg_label):
    """Exact numpy replication of the reference (quantile path included)."""
    x = np.asarray(seg_logit, dtype=np.float32)
    lbl = np.asarray(seg_label)
    Bn, Cn = x.shape[0], x.shape[1]
    xf = x.reshape(Bn, Cn, -1)
    m = xf.max(axis=1, keepdims=True)
    e = np.exp(xf - m)
    lse = np.log(e.sum(axis=1, keepdims=True)) + m
    logp = xf - lse
    l2 = np.where(lbl == IGNORE_INDEX, 0, lbl).reshape(Bn, 1, -1).astype(np.int64)
    lp_at = np.take_along_axis(logp, l2, axis=1)[:, 0]
    prob = np.exp(lp_at)
    sortp = np.sort(prob.reshape(-1))
    idx = min(MIN_KEPT * Bn, sortp.shape[0] - 1)
    thr = max(float(sortp[idx]), np.float32(0.7))
    wgt = (prob < thr).astype(np.float32)
    return np.float32((-lp_at * wgt).astype(np.float64).mean())


def _prep_core(x, lbl):
    """Per-pixel class transposition (slot0 <-> label), clamp, stage all
    slots fp8-e3m4 chunk-blocked flat (slot-major inside each chunk)."""
    xp = np.empty((C, HW), dtype=np.float32)
    xp[0] = np.take_along_axis(x, lbl[None, :], axis=0)[0]
    for k in range(1, C):
        xp[k] = np.where(lbl == k, x[0], x[k])
    np.clip(xp, -4.4, 10.0, out=xp)

    t = xp.reshape(C, P, FREE).transpose(1, 0, 2)   # [P, C, FREE]
    blocks = []
    o = 0
    for f in CHUNKS:
        blocks.append(np.ascontiguousarray(t[:, :, o : o + f]).reshape(P, C * f))
        o += f
    return np.ascontiguousarray(np.concatenate(blocks, axis=1)).astype(
        ml_dtypes.float8_e3m4
    )


def kernel(seg_logit, seg_label):
    from concourse import bass_utils

    x = np.ascontiguousarray(np.asarray(seg_logit, dtype=np.float32)).reshape(
        B, C, HW
    )
    lbl = np.asarray(seg_label)
    lbl = np.where(lbl == IGNORE_INDEX, 0, lbl).astype(np.int64).reshape(B, HW)

    ident = np.eye(P, dtype=ml_dtypes.float8_e4m3)
    id2 = np.ascontiguousarray(
        np.concatenate([ident, ident], axis=1)
    )  # [P, 2P] = [I | I]
    in_maps = []
    for b in range(B):
        in_maps.append({"logit8": _prep_core(x[b], lbl[b]), "ident2": id2})

    if "nc" not in _CACHE:
        _CACHE["nc"] = _build_nc()
    nc = _CACHE["nc"]

    res = bass_utils.run_bass_kernel_spmd(nc, in_maps, core_ids=list(range(B)))

    relu_sum = 0.0
    sign_sum = 0.0
    for r in res.results:
        a = r["acc"]
        relu_sum += float(a[:, :NGRP].sum(dtype=np.float64))
        sign_sum += float(a[:, NGRP:].sum(dtype=np.float64))

    wacc = (sign_sum + N_TOTAL) / 2.0
    if wacc <= MIN_KEPT * B:
        # quantile threshold exceeds 0.7 -> exact host path (rare/never for
        # the target distribution)
        return _host_fallback(seg_logit, seg_label)

    return np.float32((relu_sum - C0 * wacc) / N_TOTAL)
